# revision 4
# baseline (speedup 1.0000x reference)
r"""DbrxAttention on 8 TRN2 NeuronCores, tensor-parallel across heads.

Per-core shard (core c of 8): 6 query heads (q heads 6c..6c+5), kv head c
(replicated per its 6-head query group), plus the matching 768 input
columns of the out-projection. Each core computes a partial out-proj
(row-parallel Wout); the partials are summed on the host (the all-reduce
of the TP pattern).

Layouts (per core, all device tensors):
  hidT   [6144, 2048] fp16  hidden^T       (d on partitions)
  wqkvT  [6144, 1024] fp16  [q0..q5 | k | v] columns of Wqkv^T shard
  woutT  [768,  6144] fp16  Wout[:, shard]^T
  cos/sin tables [128, 2048] fp16, neox rope with sign-folded sin and the
  1/sqrt(128) score scale folded into the q tables.
  masks  [4, 128, 512] fp16  multiplicative causal masks for the four
         diagonal-straddle patterns of (128-wide kt tile, 512-wide qt chunk)

Pipeline: QKV GEMM (fp16, PSUM fp32) -> clip -> rope (DVE + partition-shift
DMA) into fp16 tiles -> scores^T = k^T.T @ q^T per (head, qt-chunk, kt-tile)
block (fp16 MM, software-pipelined 2 deep) -> exp on ACT into fp32r probs ->
causal mask multiply on diagonal blocks -> row sums via ones-matmul + attn^T
accumulation via v-matmul (both fp32r) -> normalization (reciprocal +
partition broadcast) -> fp16 attnT -> out-proj (fp16) -> partial
[2048, 6144] fp32 out, summed across the 8 cores on the host.
"""

import os

import numpy as np

import concourse.mybir as mybir
import concourse.tile as tile
from concourse import bacc
from concourse.bass_utils import run_bass_kernel_spmd

F32R = mybir.dt.float32r
F32 = mybir.dt.float32
F16 = mybir.dt.float16

T = 2048
D = 6144
N_HEADS = 48
N_KV = 8
HD = 128
CLIP = 8.0
THETA = 500000.0
N_CORES = 8
HPC = N_HEADS // N_CORES      # q heads per core = 6
QKJ = HPC + 1                 # q+k j-tiles per core = 7
DCH = D // 128                # 48 contraction chunks
TCH = T // 512                # 4 t-chunks
TTILES = T // 128             # 16 t-tiles
OCH = D // 512                # 12 out-proj column chunks
ICH = HPC                     # 6 out-proj contraction chunks (768/128)

_compiled = None


def _build():
    nc = bacc.Bacc("TRN2", target_bir_lowering=False, debug=False,
                   num_devices=N_CORES)

    hidT_d = nc.dram_tensor("hidT", [D, T], F16, kind="ExternalInput").ap()
    wqkvT_d = nc.dram_tensor("wqkvT", [D, 1024], F16, kind="ExternalInput").ap()
    woutT_d = nc.dram_tensor("woutT", [HPC * HD, D], F16, kind="ExternalInput").ap()
    cosq_d = nc.dram_tensor("cosq", [HD, T], F16, kind="ExternalInput").ap()
    sinq_d = nc.dram_tensor("sinq", [HD, T], F16, kind="ExternalInput").ap()
    cosk_d = nc.dram_tensor("cosk", [HD, T], F16, kind="ExternalInput").ap()
    sink_d = nc.dram_tensor("sink", [HD, T], F16, kind="ExternalInput").ap()
    mask_d = nc.dram_tensor("maskm", [4, HD, 512], F16, kind="ExternalInput").ap()
    ones_d = nc.dram_tensor("ones", [HD, 33], F32R, kind="ExternalInput").ap()
    outp_d = nc.dram_tensor("outp", [T, D], F32, kind="ExternalOutput").ap()

    mn, mx = mybir.AluOpType.min, mybir.AluOpType.max
    mult, add = mybir.AluOpType.mult, mybir.AluOpType.add
    EXP = mybir.ActivationFunctionType.Exp

    with tile.TileContext(nc) as tc:
        with (
            tc.tile_pool(name="sb", bufs=1) as pool,
            tc.tile_pool(name="ps", bufs=1, space="PSUM") as psum,
        ):
            # persistent tensors
            qkT = pool.tile([128, QKJ, T], F16)       # roped q (scaled) + k
            v_sb = pool.tile([128, TTILES, HD], F32R)  # clipped v, [t%128, t//128, hd]
            attnT = pool.tile([128, HPC, T], F16)      # normalized attn^T
            cosq = pool.tile([HD, T], F16)
            sinq = pool.tile([HD, T], F16)
            cosk = pool.tile([HD, T], F16)
            sink = pool.tile([HD, T], F16)
            masks = pool.tile([HD, 4, 512], F16)
            ones = pool.tile([HD, 33], F32R)

            def load_tables():
                nc.gpsimd.dma_start(cosq[:], cosq_d[:])
                nc.gpsimd.dma_start(sinq[:], sinq_d[:])
                nc.gpsimd.dma_start(cosk[:], cosk_d[:])
                nc.gpsimd.dma_start(sink[:], sink_d[:])
                nc.gpsimd.dma_start(masks[:], mask_d.rearrange("a p t -> p a t"))
                nc.gpsimd.dma_start(ones[:], ones_d[:])

            def qkv_sweep(tcx):
                tsl = slice(tcx * 512, (tcx + 1) * 512)
                qk_ps = [psum.tile([128, 512], F32, tag="bank", bufs=8,
                                   name=f"qk_ps{j}")
                         for j in range(QKJ)]
                v_ps = psum.tile([128, 512], F32, tag="bank", bufs=8)
                for d in range(DCH):
                    dsl = slice(d * 128, (d + 1) * 128)
                    hid = pool.tile([128, 512], F16, tag="hid", bufs=12)
                    wq = pool.tile([128, 1024], F16, tag="wq", bufs=12)
                    nc.sync.dma_start(hid[:], hidT_d[dsl, tsl])
                    nc.sync.dma_start(wq[:], wqkvT_d[dsl, :])
                    st, sp = d == 0, d == DCH - 1
                    for j in range(QKJ):
                        nc.tensor.matmul(qk_ps[j][:], wq[:, j * 128:(j + 1) * 128],
                                         hid[:], start=st, stop=sp)
                    for s in range(4):
                        # packed quarter-bank outputs: start=True zeroes the
                        # whole 2KB zero-region, so only the first sub-matmul
                        # of the bank may set it
                        nc.tensor.matmul(v_ps[:, s * 128:(s + 1) * 128],
                                         hid[:, s * 128:(s + 1) * 128],
                                         wq[:, 896:1024],
                                         start=(st and s == 0),
                                         stop=(sp and s == 3),
                                         skip_group_check=True)
                # evacuate: all clips first (each clip releases a PSUM bank
                # for the interleaved attention/next-sweep matmuls), ropes
                # after (they only read the SBUF raw tiles)
                raws = []
                for j in range(QKJ):
                    raw = pool.tile([128, 512], F32, tag="raw", bufs=8,
                                    name=f"raw{j}")
                    nc.vector.tensor_scalar(raw[:], qk_ps[j][:], CLIP, -CLIP, mn, mx)
                    raws.append(raw)
                nc.vector.tensor_scalar(
                    v_sb[:, tcx * 4:(tcx + 1) * 4, :],
                    v_ps[:].rearrange("p (a h) -> p a h", a=4),
                    CLIP, -CLIP, mn, mx)
                for j in [HPC] + list(range(HPC)):
                    raw = raws[j]
                    xr = pool.tile([128, 512], F32, tag="xr", bufs=4)
                    nc.sync.dma_start(xr[0:64, :], raw[64:128, :])
                    nc.sync.dma_start(xr[64:128, :], raw[0:64, :])
                    cosT = cosq if j < HPC else cosk
                    sinT = sinq if j < HPC else sink
                    dst = qkT[:, j, tsl]
                    nc.vector.tensor_tensor(dst, raw[:], cosT[:, tsl], mult)
                    nc.vector.tensor_tensor(xr[:], xr[:], sinT[:, tsl], mult)
                    nc.vector.tensor_tensor(dst, dst, xr[:], add)

            def attn_chain(h, jc):
                qsl = slice(jc * 512, (jc + 1) * 512)
                n_kt = 4 * jc + 4
                attn_ps = psum.tile([128, 512], F32, tag="bank", bufs=8)
                LEAD = 2
                pbs = {}
                # row sums accumulate on DVE (two legs to halve the serial
                # chain), then one partition-reduce matmul per chain —
                # replaces the per-kt ones-matmul (128x fewer PE rows)
                accs = [pool.tile([128, 512], F32R, tag=f"acc{i}", bufs=1,
                                  name=f"acc{i}")
                        for i in range(2)]
                for step in range(n_kt + LEAD):
                    if step < n_kt:
                        kt = step
                        sc = psum.tile([128, 512], F32, tag="bank", bufs=8)
                        nc.tensor.matmul(sc[:],
                                         qkT[:, HPC, kt * 128:(kt + 1) * 128],
                                         qkT[:, h, qsl], start=True, stop=True)
                        pb = pool.tile([128, 512], F32R, tag="pb", bufs=6)
                        nc.scalar.activation(pb[:], sc[:], EXP)
                        r = kt - 4 * jc
                        if r >= 0:
                            nc.vector.tensor_tensor(pb[:], pb[:], masks[:, r, :],
                                                    mult)
                        acc = accs[kt % 2]
                        if kt < 2:
                            nc.vector.tensor_scalar(acc[:], pb[:], 0.0, None, add)
                        else:
                            nc.vector.tensor_tensor(acc[:], acc[:], pb[:], add)
                        pbs[kt] = pb
                    if step >= LEAD:
                        kt = step - LEAD
                        pb = pbs.pop(kt)
                        st, sp = kt == 0, kt == n_kt - 1
                        nc.tensor.matmul(attn_ps[:], v_sb[:, kt, :], pb[:],
                                         start=st, stop=sp)
                nc.vector.tensor_tensor(accs[0][:], accs[0][:], accs[1][:], add)
                sums_ps = psum.tile([1, 512], F32, tag="bank", bufs=8)
                nc.tensor.matmul(sums_ps[:], ones[:, 0:1], accs[0][:],
                                 start=True, stop=True)
                # release the bank fast (ACT copy + DVE recip), then
                # normalize off the critical path
                au = pool.tile([128, 512], F32, tag="au", bufs=4)
                nc.scalar.copy(au[:], attn_ps[:])
                rec = pool.tile([1, 512], F32, tag="rec", bufs=4)
                nc.vector.reciprocal(rec[:], sums_ps[:])
                recb = pool.tile([128, 512], F32, tag="recb", bufs=4)
                nc.gpsimd.partition_broadcast(recb[:], rec[:])
                nc.vector.tensor_tensor(attnT[:, h, qsl], au[:], recb[:], mult)

            def outproj():
                for oc in range(OCH):
                    osl = slice(oc * 512, (oc + 1) * 512)
                    wo = pool.tile([128, ICH, 512], F16, tag="wo", bufs=3)
                    nc.sync.dma_start(wo[:], woutT_d[:, osl].rearrange(
                        "(i p) o -> p i o", p=128))
                    for t in range(TTILES):
                        out_ps = psum.tile([128, 512], F32, tag="bank", bufs=8)
                        for i in range(ICH):
                            nc.tensor.matmul(out_ps[:],
                                             attnT[:, i, t * 128:(t + 1) * 128],
                                             wo[:, i, :], start=(i == 0),
                                             stop=(i == ICH - 1))
                        osb = pool.tile([128, 512], F32, tag="osb", bufs=4)
                        nc.scalar.copy(osb[:], out_ps[:])
                        nc.sync.dma_start(outp_d[t * 128:(t + 1) * 128, osl], osb[:])

            # ---- Sequential phases; chains jc-outer so the last sweep's
            # rope only gates the final quarter of chains ----
            load_tables()
            for tcx in range(TCH):
                qkv_sweep(tcx)
            for jc in range(TCH):
                for h in range(HPC):
                    attn_chain(h, jc)
            outproj()

    nc.compile()
    return nc


def kernel(hidden_states, position_ids, Wqkv, Wout):
    global _compiled
    hidden_states = np.asarray(hidden_states, dtype=np.float32)
    position_ids = np.asarray(position_ids).astype(np.int64)
    Wqkv = np.asarray(Wqkv, dtype=np.float32)
    Wout = np.asarray(Wout, dtype=np.float32)

    if _compiled is None:
        _compiled = _build()
    nc = _compiled

    # host prep: rope tables (from actual position_ids), masks, shards
    scale = HD ** -0.5
    half = HD // 2
    inv_freq = 1.0 / (THETA ** (np.arange(half, dtype=np.float64) / half))
    freqs = position_ids.astype(np.float64)[None, :] * inv_freq[:, None]  # [64, T]
    cos = np.cos(freqs)
    sin = np.sin(freqs)
    cosf = np.concatenate([cos, cos], 0)
    sinf = np.concatenate([-sin, sin], 0)
    cosq = (cosf * scale).astype(np.float16)
    sinq = (sinf * scale).astype(np.float16)
    cosk = cosf.astype(np.float16)
    sink = sinf.astype(np.float16)

    p = np.arange(128)[:, None]
    f = np.arange(512)[None, :]
    masks = np.stack([(f >= 128 * r + p) for r in range(4)]).astype(np.float16)

    hidT = np.ascontiguousarray(hidden_states.T).astype(np.float16)
    ones = np.ones((HD, 33), np.float32)

    q_size = N_HEADS * HD
    in_maps = []
    for c in range(N_CORES):
        qrows = Wqkv[c * HPC * HD:(c + 1) * HPC * HD]
        krows = Wqkv[q_size + c * HD:q_size + (c + 1) * HD]
        vrows = Wqkv[q_size + N_KV * HD + c * HD:q_size + N_KV * HD + (c + 1) * HD]
        wqkvT = np.ascontiguousarray(
            np.concatenate([qrows, krows, vrows], 0).T).astype(np.float16)
        woutT = np.ascontiguousarray(
            Wout[:, c * HPC * HD:(c + 1) * HPC * HD].T).astype(np.float16)
        in_maps.append({
            "hidT": hidT, "wqkvT": wqkvT, "woutT": woutT,
            "cosq": cosq, "sinq": sinq, "cosk": cosk, "sink": sink,
            "maskm": masks, "ones": ones,
        })

    trace = os.environ.get("DBRX_TRACE", "0") == "1"
    res = run_bass_kernel_spmd(nc, in_maps, core_ids=list(range(N_CORES)),
                               trace=trace)
    kernel.last_result = res

    out = res.results[0]["outp"].astype(np.float32)
    for c in range(1, N_CORES):
        out += res.results[c]["outp"]
    return out



# revision 28
# speedup vs baseline: 1.0626x; 1.0626x over previous
r"""DbrxAttention on 8 TRN2 NeuronCores, tensor-parallel across heads.

Per-core shard (core c of 8): 6 query heads (q heads 6c..6c+5), kv head c
(replicated per its 6-head query group), plus the matching 768 input
columns of the out-projection. Each core computes a partial out-proj
(row-parallel Wout); the partials are summed on the host (the all-reduce
of the TP pattern).

Layouts (per core, all device tensors):
  hidT   [6144, 2048] fp16  hidden^T       (d on partitions)
  wqkvT  [6144, 1024] fp16  [q0..q5 | k | v] columns of Wqkv^T shard
  woutT  [768,  6144] fp16  Wout[:, shard]^T
  cos/sin tables [128, 2048] fp16, neox rope with sign-folded sin and the
  1/sqrt(128) score scale folded into the q tables.
  masks  [4, 128, 512] fp16  multiplicative causal masks for the four
         diagonal-straddle patterns of (128-wide kt tile, 512-wide qt chunk)

Pipeline: QKV GEMM (fp16, PSUM fp32) -> clip -> rope (DVE + partition-shift
DMA) into fp16 tiles -> scores^T = k^T.T @ q^T per (head, qt-chunk, kt-tile)
block (fp16 MM, software-pipelined 2 deep) -> exp on ACT into fp32r probs ->
causal mask multiply on diagonal blocks -> softmax row sums accumulated on
DVE (two legs) + one partition-reduce matmul per chain -> attn^T via
v-matmul (fp32r) -> normalization (reciprocal + partition broadcast) ->
fp16 attnT -> out-proj (fp16) interleaved per 4-t-tile group right after
its chains (fills PE while exp paces the chains) -> partial [2048, 6144]
fp16 out, summed across the 8 cores on the host.
"""

import os

import numpy as np

import concourse.mybir as mybir
import concourse.tile as tile
from concourse import bacc
from concourse.bass_utils import run_bass_kernel_spmd

F32R = mybir.dt.float32r
F32 = mybir.dt.float32
F16 = mybir.dt.float16
BF16 = mybir.dt.bfloat16

T = 2048
D = 6144
N_HEADS = 48
N_KV = 8
HD = 128
CLIP = 8.0
THETA = 500000.0
N_CORES = 8
HPC = N_HEADS // N_CORES      # q heads per core = 6
QKJ = HPC + 1                 # q+k j-tiles per core = 7
DCH = D // 128                # 48 contraction chunks
DG = DCH // 4                 # 12 batched (4-chunk) DMA groups
TCH = T // 512                # 4 t-chunks
TTILES = T // 128             # 16 t-tiles
OCH = D // 512                # 12 out-proj column chunks
ICH = HPC                     # 6 out-proj contraction chunks (768/128)

_compiled = None


def _build():
    nc = bacc.Bacc("TRN2", target_bir_lowering=False, debug=False,
                   num_devices=N_CORES)

    hidT_d = nc.dram_tensor("hidT", [D, T], F16, kind="ExternalInput").ap()
    wqkvT_d = nc.dram_tensor("wqkvT", [D, 1024], F16, kind="ExternalInput").ap()
    woutT_d = nc.dram_tensor("woutT", [HPC * HD, D], F16, kind="ExternalInput").ap()
    cosq_d = nc.dram_tensor("cosq", [HD, T], F16, kind="ExternalInput").ap()
    sinq_d = nc.dram_tensor("sinq", [HD, T], F16, kind="ExternalInput").ap()
    cosk_d = nc.dram_tensor("cosk", [HD, T], F16, kind="ExternalInput").ap()
    sink_d = nc.dram_tensor("sink", [HD, T], F16, kind="ExternalInput").ap()
    mask_d = nc.dram_tensor("maskm", [4, HD, 512], F16, kind="ExternalInput").ap()
    outp_d = nc.dram_tensor("outp", [T, D], F16, kind="ExternalOutput").ap()

    mn, mx = mybir.AluOpType.min, mybir.AluOpType.max
    mult, add = mybir.AluOpType.mult, mybir.AluOpType.add
    EXP = mybir.ActivationFunctionType.Exp

    with tile.TileContext(nc) as tc:
        with (
            tc.tile_pool(name="sb", bufs=1) as pool,
            tc.tile_pool(name="ps", bufs=1, space="PSUM") as psum,
        ):
            # persistent tensors
            qkT = pool.tile([128, QKJ, T], F16)       # roped q (scaled) + k
            v_sb = pool.tile([128, TTILES, HD], BF16)  # clipped v, [t%128, t//128, hd]
            attnT = pool.tile([128, HPC, T], F16)      # normalized attn^T
            cosq = pool.tile([HD, T], F16)
            sinq = pool.tile([HD, T], F16)
            cosk = pool.tile([HD, T], F16)
            sink = pool.tile([HD, T], F16)
            masks = pool.tile([HD, 4, 512], F16)
            ones = pool.tile([HD, 1], BF16)

            def load_tables():
                nc.gpsimd.dma_start(cosq[:], cosq_d[:])
                nc.gpsimd.dma_start(sinq[:], sinq_d[:])
                nc.gpsimd.dma_start(cosk[:], cosk_d[:])
                nc.gpsimd.dma_start(sink[:], sink_d[:])
                nc.gpsimd.dma_start(masks[:], mask_d.rearrange("a p t -> p a t"))
                nc.vector.memset(ones[:], 1.0)

            def qkv_sweep(tcx):
                tsl = slice(tcx * 512, (tcx + 1) * 512)
                # PSUM: two 2-bank "wide" tiles hold j-pairs (j01, j23);
                # j4..j6 and v use the 1-bank "bank" ring — 8 banks total,
                # same live set as before but shaped so chains can use a
                # 2-bank sc2 (single 1024-wide exp per 2 kt tiles)
                wides = [psum.tile([128, 1024], F32, tag="wide", bufs=2,
                                   name=f"qkw{w}") for w in range(2)]
                qk_ps = [wides[j // 2][:, (j % 2) * 512:(j % 2 + 1) * 512]
                         for j in range(4)]
                qk_ps += [psum.tile([128, 512], F32, tag="bank", bufs=4,
                                    name=f"qk_ps{j}")[:]
                          for j in range(4, QKJ)]
                v_ps = psum.tile([128, 512], F32, tag="bank", bufs=4)
                for g in range(DG):
                    # batched 4-chunk loads: 4x fewer DMAs on the sync queue
                    g4 = slice(g * 512, (g + 1) * 512)
                    hid4 = pool.tile([128, 4, 512], F16, tag="hid", bufs=3)
                    wq4 = pool.tile([128, 4, 1024], F16, tag="wq", bufs=3)
                    nc.sync.dma_start(
                        hid4[:], hidT_d[g4, tsl].rearrange("(a p) t -> p a t",
                                                           p=128))
                    nc.sync.dma_start(
                        wq4[:], wqkvT_d[g4, :].rearrange("(a p) w -> p a w",
                                                         p=128))
                    for i in range(4):
                        d = g * 4 + i
                        hid = hid4[:, i, :]
                        wq = wq4[:, i, :]
                        st, sp = d == 0, d == DCH - 1
                        for j in range(QKJ):
                            nc.tensor.matmul(qk_ps[j],
                                             wq[:, j * 128:(j + 1) * 128],
                                             hid[:], start=st, stop=sp,
                                             skip_group_check=(j < 4))
                        for s in range(4):
                            # packed quarter-bank outputs: start=True zeroes
                            # the whole 2KB zero-region, so only the first
                            # sub-matmul of the bank may set it
                            nc.tensor.matmul(v_ps[:, s * 128:(s + 1) * 128],
                                             hid[:, s * 128:(s + 1) * 128],
                                             wq[:, 896:1024],
                                             start=(st and s == 0),
                                             stop=(sp and s == 3),
                                             skip_group_check=True)
                # evacuate: all clips first (each clip releases a PSUM bank
                # for the interleaved attention/next-sweep matmuls), ropes
                # after (they only read the SBUF raw tiles)
                raws = []
                for w in range(2):
                    raw2 = pool.tile([128, 1024], F32, tag="raw2", bufs=3,
                                     name=f"raw2_{w}")
                    nc.vector.tensor_scalar(raw2[:], wides[w][:], CLIP, -CLIP,
                                            mn, mx)
                    raws += [raw2[:, 0:512], raw2[:, 512:1024]]
                for j in range(4, QKJ):
                    raw = pool.tile([128, 512], F32, tag="raw", bufs=4,
                                    name=f"raw{j}")
                    nc.vector.tensor_scalar(raw[:], qk_ps[j], CLIP, -CLIP, mn, mx)
                    raws.append(raw[:])
                nc.vector.tensor_scalar(
                    v_sb[:, tcx * 4:(tcx + 1) * 4, :],
                    v_ps[:].rearrange("p (a h) -> p a h", a=4),
                    CLIP, -CLIP, mn, mx)
                # ropes run on POOL: keeps them out of the DVE stream so the
                # first chains' masks/accs aren't head-of-line blocked behind
                # the last sweep's rope tail
                for j in [HPC] + list(range(HPC)):
                    raw = raws[j]
                    xr = pool.tile([128, 512], F32, tag="xr", bufs=4)
                    nc.sync.dma_start(xr[0:64, :], raw[64:128, :])
                    nc.sync.dma_start(xr[64:128, :], raw[0:64, :])
                    cosT = cosq if j < HPC else cosk
                    sinT = sinq if j < HPC else sink
                    dst = qkT[:, j, tsl]
                    nc.gpsimd.tensor_tensor(dst, raw, cosT[:, tsl], mult)
                    nc.gpsimd.tensor_tensor(xr[:], xr[:], sinT[:, tsl], mult)
                    nc.gpsimd.tensor_tensor(dst, dst, xr[:], add)

            def attn_chain(h, jc):
                # generator: yields once per 2-kt block so the driver can
                # interleave ready out-proj matmuls into the in-order PE
                # stream (fills the PE bubble left by the ACT-paced exp).
                # Scores for a kt-pair land in one 2-bank "wide" PSUM tile so
                # a single 1024-wide exp serves both (less ACT overhead).
                qsl = slice(jc * 512, (jc + 1) * 512)
                n_kt = 4 * jc + 4
                n_b = n_kt // 2
                attn_ps = psum.tile([128, 512], F32, tag="bank", bufs=4)
                LEAD = 1
                pbs = {}
                # row sums accumulate on DVE in bf16 (2-byte dtype gets the
                # fast DVE mode; 2 legs bound partial-sum rounding and the
                # serial chain), one partition-reduce matmul per chain —
                # replaces the per-kt ones-matmul (128x fewer PE rows)
                accs = [pool.tile([128, 512], BF16, tag=f"acc{i}", bufs=1,
                                  name=f"acc{i}")
                        for i in range(2)]
                for bstep in range(n_b + LEAD):
                    if bstep < n_b:
                        b = bstep
                        sc2 = psum.tile([128, 1024], F32, tag="wide", bufs=2)
                        for half in range(2):
                            kt = 2 * b + half
                            nc.tensor.matmul(
                                sc2[:, half * 512:(half + 1) * 512],
                                qkT[:, HPC, kt * 128:(kt + 1) * 128],
                                qkT[:, h, qsl], start=True, stop=True,
                                skip_group_check=True)
                        pb2 = pool.tile([128, 1024], BF16, tag="pb", bufs=4)
                        nc.scalar.activation(pb2[:], sc2[:], EXP)
                        for half in range(2):
                            kt = 2 * b + half
                            r = kt - 4 * jc
                            if r >= 0:
                                nc.vector.tensor_tensor(
                                    pb2[:, half * 512:(half + 1) * 512],
                                    pb2[:, half * 512:(half + 1) * 512],
                                    masks[:, r, :], mult)
                        acc = accs[b % 2]
                        if b < 2:
                            nc.vector.tensor_tensor(acc[:], pb2[:, 0:512],
                                                    pb2[:, 512:1024], add)
                        else:
                            with nc.allow_low_precision(
                                    reason="bf16 row-sum legs, 2e-2 budget"):
                                nc.vector.tensor_tensor(acc[:], acc[:],
                                                        pb2[:, 0:512], add)
                                nc.vector.tensor_tensor(acc[:], acc[:],
                                                        pb2[:, 512:1024], add)
                        pbs[b] = pb2
                    if bstep >= LEAD:
                        b = bstep - LEAD
                        pb2 = pbs.pop(b)
                        for half in range(2):
                            kt = 2 * b + half
                            st, sp = kt == 0, kt == n_kt - 1
                            nc.tensor.matmul(
                                attn_ps[:], v_sb[:, kt, :],
                                pb2[:, half * 512:(half + 1) * 512],
                                start=st, stop=sp)
                    yield
                with nc.allow_low_precision(
                        reason="bf16 row-sum combine, 2e-2 budget"):
                    nc.vector.tensor_tensor(accs[0][:], accs[0][:], accs[1][:],
                                            add)
                sums_ps = psum.tile([1, 512], F32, tag="bank", bufs=4)
                nc.tensor.matmul(sums_ps[:], ones[:, 0:1], accs[0][:],
                                 start=True, stop=True)
                # release the bank fast (ACT copy + DVE recip), then
                # normalize off the critical path; all-bf16 so the norm
                # multiply gets the fast DVE mode
                au = pool.tile([128, 512], BF16, tag="au", bufs=4)
                nc.scalar.copy(au[:], attn_ps[:])
                rec = pool.tile([1, 512], BF16, tag="rec", bufs=4)
                with nc.allow_low_precision(
                        reason="bf16 softmax scale, 2e-2 budget"):
                    nc.vector.reciprocal(rec[:], sums_ps[:])
                recb = pool.tile([128, 512], BF16, tag="recb", bufs=4)
                nc.gpsimd.partition_broadcast(recb[:], rec[:])
                nc.vector.tensor_tensor(attnT[:, h, qsl], au[:], recb[:], mult)

            def outproj_blocks(jc):
                # generator of out-proj (oc, tt) blocks for t-tiles
                # 4jc..4jc+3; drained one block per chain step of the NEXT
                # group so PE never idles while exp paces the chains.
                # PSUM->SBUF copies run on POOL (ACT would head-of-line
                # block the next group's exps)
                tg = slice(jc * 512, (jc + 1) * 512)
                for oc in range(OCH):
                    osl = slice(oc * 512, (oc + 1) * 512)
                    wo = pool.tile([128, ICH, 512], F16, tag="wo", bufs=3)
                    nc.sync.dma_start(wo[:], woutT_d[:, osl].rearrange(
                        "(i p) o -> p i o", p=128))
                    osb = pool.tile([128, 4, 512], F16, tag="osb", bufs=2)
                    for tt in range(4):
                        t = 4 * jc + tt
                        out_ps = psum.tile([128, 512], F32, tag="bank", bufs=4)
                        for i in range(ICH):
                            nc.tensor.matmul(out_ps[:],
                                             attnT[:, i, t * 128:(t + 1) * 128],
                                             wo[:, i, :], start=(i == 0),
                                             stop=(i == ICH - 1))
                        # PSUM->SBUF evac: GPSIMD cannot read PSUM, so
                        # alternate ACT/DVE to halve per-engine load
                        if (oc + tt) % 2 == 0:
                            nc.scalar.copy(osb[:, tt, :], out_ps[:])
                        else:
                            nc.vector.tensor_copy(osb[:, tt, :], out_ps[:])
                        yield
                    nc.sync.dma_start(
                        outp_d[tg, osl].rearrange("(a p) o -> p a o", p=128),
                        osb[:])

            # ---- QKV sweeps, then chain groups with the previous group's
            # out-proj blocks interleaved one-per-step into the PE stream ----
            load_tables()
            for tcx in range(TCH):
                qkv_sweep(tcx)
            op_gen = None
            for jc in range(TCH):
                # spread the previous group's 48 out-proj blocks evenly
                # across this group's chain steps
                steps = HPC * ((4 * jc + 4) // 2 + 1)
                rate = (4 * OCH) / steps if op_gen is not None else 0.0
                due = 0.0
                for h in range(HPC):
                    for _ in attn_chain(h, jc):
                        due += rate
                        while due >= 1.0:
                            next(op_gen, None)
                            due -= 1.0
                if op_gen is not None:
                    for _ in op_gen:
                        pass
                op_gen = outproj_blocks(jc)
            for _ in op_gen:
                pass

    nc.compile()
    return nc


def kernel(hidden_states, position_ids, Wqkv, Wout):
    global _compiled
    hidden_states = np.asarray(hidden_states, dtype=np.float32)
    position_ids = np.asarray(position_ids).astype(np.int64)
    Wqkv = np.asarray(Wqkv, dtype=np.float32)
    Wout = np.asarray(Wout, dtype=np.float32)

    if _compiled is None:
        _compiled = _build()
    nc = _compiled

    # host prep: rope tables (from actual position_ids), masks, shards
    scale = HD ** -0.5
    half = HD // 2
    inv_freq = 1.0 / (THETA ** (np.arange(half, dtype=np.float64) / half))
    freqs = position_ids.astype(np.float64)[None, :] * inv_freq[:, None]  # [64, T]
    cos = np.cos(freqs)
    sin = np.sin(freqs)
    cosf = np.concatenate([cos, cos], 0)
    sinf = np.concatenate([-sin, sin], 0)
    cosq = (cosf * scale).astype(np.float16)
    sinq = (sinf * scale).astype(np.float16)
    cosk = cosf.astype(np.float16)
    sink = sinf.astype(np.float16)

    p = np.arange(128)[:, None]
    f = np.arange(512)[None, :]
    masks = np.stack([(f >= 128 * r + p) for r in range(4)]).astype(np.float16)

    hidT = np.ascontiguousarray(hidden_states.T).astype(np.float16)

    q_size = N_HEADS * HD
    in_maps = []
    for c in range(N_CORES):
        qrows = Wqkv[c * HPC * HD:(c + 1) * HPC * HD]
        krows = Wqkv[q_size + c * HD:q_size + (c + 1) * HD]
        vrows = Wqkv[q_size + N_KV * HD + c * HD:q_size + N_KV * HD + (c + 1) * HD]
        wqkvT = np.ascontiguousarray(
            np.concatenate([qrows, krows, vrows], 0).T).astype(np.float16)
        woutT = np.ascontiguousarray(
            Wout[:, c * HPC * HD:(c + 1) * HPC * HD].T).astype(np.float16)
        in_maps.append({
            "hidT": hidT, "wqkvT": wqkvT, "woutT": woutT,
            "cosq": cosq, "sinq": sinq, "cosk": cosk, "sink": sink,
            "maskm": masks,
        })

    trace = os.environ.get("DBRX_TRACE", "0") == "1"
    res = run_bass_kernel_spmd(nc, in_maps, core_ids=list(range(N_CORES)),
                               trace=trace)
    kernel.last_result = res

    out = res.results[0]["outp"].astype(np.float32)
    for c in range(1, N_CORES):
        out += res.results[c]["outp"].astype(np.float32)
    return out


# revision 29
# speedup vs baseline: 1.0844x; 1.0205x over previous
r"""DbrxAttention on 8 TRN2 NeuronCores, tensor-parallel across heads.

Per-core shard (core c of 8): 6 query heads (q heads 6c..6c+5), kv head c
(replicated per its 6-head query group), plus the matching 768 input
columns of the out-projection. Each core computes a partial out-proj
(row-parallel Wout); the partials are summed on the host (the all-reduce
of the TP pattern).

Layouts (per core, all device tensors):
  hidT   [6144, 2048] fp16  hidden^T       (d on partitions)
  wqkvT  [6144, 1024] fp16  [q0..q5 | k | v] columns of Wqkv^T shard
  woutT  [768,  6144] fp16  Wout[:, shard]^T
  cos/sin tables [128, 2048] fp16, neox rope with sign-folded sin and the
  1/sqrt(128) score scale folded into the q tables.
  masks  [4, 128, 512] fp16  multiplicative causal masks for the four
         diagonal-straddle patterns of (128-wide kt tile, 512-wide qt chunk)

Pipeline: QKV GEMM (fp16, PSUM fp32) -> clip -> rope (DVE + partition-shift
DMA) into fp16 tiles -> scores^T = k^T.T @ q^T per (head, qt-chunk, kt-tile)
block (fp16 MM, software-pipelined 2 deep) -> exp on ACT into fp32r probs ->
causal mask multiply on diagonal blocks -> softmax row sums accumulated on
DVE (two legs) + one partition-reduce matmul per chain -> attn^T via
v-matmul (fp32r) -> normalization (reciprocal + partition broadcast) ->
fp16 attnT -> out-proj (fp16) interleaved per 4-t-tile group right after
its chains (fills PE while exp paces the chains) -> partial [2048, 6144]
fp16 out, summed across the 8 cores on the host.
"""

import os

import numpy as np

import concourse.mybir as mybir
import concourse.tile as tile
from concourse import bacc
from concourse.bass_utils import run_bass_kernel_spmd

F32R = mybir.dt.float32r
F32 = mybir.dt.float32
F16 = mybir.dt.float16
BF16 = mybir.dt.bfloat16

T = 2048
D = 6144
N_HEADS = 48
N_KV = 8
HD = 128
CLIP = 8.0
THETA = 500000.0
N_CORES = 8
HPC = N_HEADS // N_CORES      # q heads per core = 6
QKJ = HPC + 1                 # q+k j-tiles per core = 7
DCH = D // 128                # 48 contraction chunks
DG = DCH // 4                 # 12 batched (4-chunk) DMA groups
TCH = T // 512                # 4 t-chunks
TTILES = T // 128             # 16 t-tiles
OCH = D // 512                # 12 out-proj column chunks
ICH = HPC                     # 6 out-proj contraction chunks (768/128)

_compiled = None


def _build():
    nc = bacc.Bacc("TRN2", target_bir_lowering=False, debug=False,
                   num_devices=N_CORES)

    hidT_d = nc.dram_tensor("hidT", [D, T], F16, kind="ExternalInput").ap()
    wqkvT_d = nc.dram_tensor("wqkvT", [D, 1024], F16, kind="ExternalInput").ap()
    woutT_d = nc.dram_tensor("woutT", [HPC * HD, D], F16, kind="ExternalInput").ap()
    cosq_d = nc.dram_tensor("cosq", [HD, T], F16, kind="ExternalInput").ap()
    sinq_d = nc.dram_tensor("sinq", [HD, T], F16, kind="ExternalInput").ap()
    cosk_d = nc.dram_tensor("cosk", [HD, T], F16, kind="ExternalInput").ap()
    sink_d = nc.dram_tensor("sink", [HD, T], F16, kind="ExternalInput").ap()
    mask_d = nc.dram_tensor("maskm", [HD, 128], F16, kind="ExternalInput").ap()
    outp_d = nc.dram_tensor("outp", [T, D], F16, kind="ExternalOutput").ap()

    mn, mx = mybir.AluOpType.min, mybir.AluOpType.max
    mult, add = mybir.AluOpType.mult, mybir.AluOpType.add
    EXP = mybir.ActivationFunctionType.Exp

    with tile.TileContext(nc) as tc:
        with (
            tc.tile_pool(name="sb", bufs=1) as pool,
            tc.tile_pool(name="ps", bufs=1, space="PSUM") as psum,
        ):
            # persistent tensors
            qkT = pool.tile([128, QKJ, T], F16)       # roped q (scaled) + k
            v_sb = pool.tile([128, TTILES, HD], BF16)  # clipped v, [t%128, t//128, hd]
            attnT = pool.tile([128, HPC, T], F16)      # normalized attn^T
            cosq = pool.tile([HD, T], F16)
            sinq = pool.tile([HD, T], F16)
            cosk = pool.tile([HD, T], F16)
            sink = pool.tile([HD, T], F16)
            masks = pool.tile([HD, 128], F16)
            ones = pool.tile([HD, 1], BF16)

            def load_tables():
                nc.gpsimd.dma_start(cosq[:], cosq_d[:])
                nc.gpsimd.dma_start(sinq[:], sinq_d[:])
                nc.gpsimd.dma_start(cosk[:], cosk_d[:])
                nc.gpsimd.dma_start(sink[:], sink_d[:])
                nc.gpsimd.dma_start(masks[:], mask_d[:])
                nc.vector.memset(ones[:], 1.0)

            def qkv_sweep(tcx):
                tsl = slice(tcx * 512, (tcx + 1) * 512)
                # PSUM: two 2-bank "wide" tiles hold j-pairs (j01, j23);
                # j4..j6 and v use the 1-bank "bank" ring — 8 banks total,
                # same live set as before but shaped so chains can use a
                # 2-bank sc2 (single 1024-wide exp per 2 kt tiles)
                wides = [psum.tile([128, 1024], F32, tag="wide", bufs=2,
                                   name=f"qkw{w}") for w in range(2)]
                qk_ps = [wides[j // 2][:, (j % 2) * 512:(j % 2 + 1) * 512]
                         for j in range(4)]
                qk_ps += [psum.tile([128, 512], F32, tag="bank", bufs=4,
                                    name=f"qk_ps{j}")[:]
                          for j in range(4, QKJ)]
                v_ps = psum.tile([128, 512], F32, tag="bank", bufs=4)
                for g in range(DG):
                    # batched 4-chunk loads: 4x fewer DMAs on the sync queue
                    g4 = slice(g * 512, (g + 1) * 512)
                    hid4 = pool.tile([128, 4, 512], F16, tag="hid", bufs=3)
                    wq4 = pool.tile([128, 4, 1024], F16, tag="wq", bufs=3)
                    nc.sync.dma_start(
                        hid4[:], hidT_d[g4, tsl].rearrange("(a p) t -> p a t",
                                                           p=128))
                    nc.sync.dma_start(
                        wq4[:], wqkvT_d[g4, :].rearrange("(a p) w -> p a w",
                                                         p=128))
                    for i in range(4):
                        d = g * 4 + i
                        hid = hid4[:, i, :]
                        wq = wq4[:, i, :]
                        st, sp = d == 0, d == DCH - 1
                        for j in range(QKJ):
                            nc.tensor.matmul(qk_ps[j],
                                             wq[:, j * 128:(j + 1) * 128],
                                             hid[:], start=st, stop=sp,
                                             skip_group_check=(j < 4))
                        for s in range(4):
                            # packed quarter-bank outputs: start=True zeroes
                            # the whole 2KB zero-region, so only the first
                            # sub-matmul of the bank may set it
                            nc.tensor.matmul(v_ps[:, s * 128:(s + 1) * 128],
                                             hid[:, s * 128:(s + 1) * 128],
                                             wq[:, 896:1024],
                                             start=(st and s == 0),
                                             stop=(sp and s == 3),
                                             skip_group_check=True)
                # evacuate: all clips first (each clip releases a PSUM bank
                # for the interleaved attention/next-sweep matmuls), ropes
                # after (they only read the SBUF raw tiles)
                raws = []
                for w in range(2):
                    raw2 = pool.tile([128, 1024], F32, tag="raw2", bufs=3,
                                     name=f"raw2_{w}")
                    nc.vector.tensor_scalar(raw2[:], wides[w][:], CLIP, -CLIP,
                                            mn, mx)
                    raws += [raw2[:, 0:512], raw2[:, 512:1024]]
                for j in range(4, QKJ):
                    raw = pool.tile([128, 512], F32, tag="raw", bufs=4,
                                    name=f"raw{j}")
                    nc.vector.tensor_scalar(raw[:], qk_ps[j], CLIP, -CLIP, mn, mx)
                    raws.append(raw[:])
                nc.vector.tensor_scalar(
                    v_sb[:, tcx * 4:(tcx + 1) * 4, :],
                    v_ps[:].rearrange("p (a h) -> p a h", a=4),
                    CLIP, -CLIP, mn, mx)
                # ropes run on POOL: keeps them out of the DVE stream so the
                # first chains' masks/accs aren't head-of-line blocked behind
                # the last sweep's rope tail
                for j in [HPC] + list(range(HPC)):
                    raw = raws[j]
                    xr = pool.tile([128, 512], F32, tag="xr", bufs=4)
                    nc.sync.dma_start(xr[0:64, :], raw[64:128, :])
                    nc.sync.dma_start(xr[64:128, :], raw[0:64, :])
                    cosT = cosq if j < HPC else cosk
                    sinT = sinq if j < HPC else sink
                    dst = qkT[:, j, tsl]
                    nc.gpsimd.tensor_tensor(dst, raw, cosT[:, tsl], mult)
                    nc.gpsimd.tensor_tensor(xr[:], xr[:], sinT[:, tsl], mult)
                    nc.gpsimd.tensor_tensor(dst, dst, xr[:], add)

            def attn_chain(h, jc):
                # generator: yields once per 2-kt block so the driver can
                # interleave ready out-proj matmuls into the in-order PE
                # stream (fills the PE bubble left by the ACT-paced exp).
                # Scores for a kt-pair land in one 2-bank "wide" PSUM tile so
                # a single 1024-wide exp serves both (less ACT overhead).
                # Diagonal-straddle kt tiles (r = kt-4jc >= 0) compute only
                # the causally-needed q-suffix [128r:512] — 15% less
                # score/v PE work; the in-tile triangle is masked by one
                # [128,128] pattern at the suffix head. The unwritten prefix
                # of those PSUM halves holds stale data; exp covers it but
                # nothing downstream reads it.
                qsl = slice(jc * 512, (jc + 1) * 512)
                n_kt = 4 * jc + 4
                n_b = n_kt // 2
                attn_ps = psum.tile([128, 512], F32, tag="bank", bufs=4)
                LEAD = 1
                pbs = {}
                # row sums accumulate on DVE in bf16 (2-byte dtype gets the
                # fast DVE mode); suffix-kt adds land in leg 0 (always fully
                # initialized by kt 0), full-width kts alternate legs
                accs = [pool.tile([128, 512], BF16, tag=f"acc{i}", bufs=1,
                                  name=f"acc{i}")
                        for i in range(2)]
                two_legs = jc >= 1
                for bstep in range(n_b + LEAD):
                    if bstep < n_b:
                        b = bstep
                        sc2 = psum.tile([128, 1024], F32, tag="wide", bufs=2)
                        for half in range(2):
                            kt = 2 * b + half
                            r = kt - 4 * jc
                            off = 128 * r if r > 0 else 0
                            nc.tensor.matmul(
                                sc2[:, half * 512 + off:(half + 1) * 512],
                                qkT[:, HPC, kt * 128:(kt + 1) * 128],
                                qkT[:, h, jc * 512 + off:(jc + 1) * 512],
                                start=True, stop=True,
                                skip_group_check=True)
                        pb2 = pool.tile([128, 1024], BF16, tag="pb", bufs=4)
                        nc.scalar.activation(pb2[:], sc2[:], EXP)
                        for half in range(2):
                            kt = 2 * b + half
                            r = kt - 4 * jc
                            if r >= 0:
                                msl = slice(half * 512 + 128 * r,
                                            half * 512 + 128 * r + 128)
                                nc.vector.tensor_tensor(
                                    pb2[:, msl], pb2[:, msl], masks[:], mult)
                        for half in range(2):
                            kt = 2 * b + half
                            r = kt - 4 * jc
                            if r > 0:
                                with nc.allow_low_precision(
                                        reason="bf16 row-sum legs"):
                                    nc.vector.tensor_tensor(
                                        accs[0][:, 128 * r:512],
                                        accs[0][:, 128 * r:512],
                                        pb2[:, half * 512 + 128 * r:
                                            (half + 1) * 512], add)
                            else:
                                leg = accs[kt % 2] if two_legs else accs[0]
                                psl = pb2[:, half * 512:(half + 1) * 512]
                                if kt < 2:
                                    nc.vector.tensor_scalar(
                                        leg[:], psl, 0.0, None, add)
                                else:
                                    with nc.allow_low_precision(
                                            reason="bf16 row-sum legs"):
                                        nc.vector.tensor_tensor(
                                            leg[:], leg[:], psl, add)
                        pbs[b] = pb2
                    if bstep >= LEAD:
                        b = bstep - LEAD
                        pb2 = pbs.pop(b)
                        for half in range(2):
                            kt = 2 * b + half
                            r = kt - 4 * jc
                            off = 128 * r if r > 0 else 0
                            st, sp = kt == 0, kt == n_kt - 1
                            nc.tensor.matmul(
                                attn_ps[:, off:512], v_sb[:, kt, :],
                                pb2[:, half * 512 + off:(half + 1) * 512],
                                start=st, stop=sp, skip_group_check=True)
                    yield
                if two_legs:
                    with nc.allow_low_precision(
                            reason="bf16 row-sum combine, 2e-2 budget"):
                        nc.vector.tensor_tensor(accs[0][:], accs[0][:],
                                                accs[1][:], add)
                sums_ps = psum.tile([1, 512], F32, tag="bank", bufs=4)
                nc.tensor.matmul(sums_ps[:], ones[:, 0:1], accs[0][:],
                                 start=True, stop=True)
                # release the bank fast (ACT copy + DVE recip), then
                # normalize off the critical path; all-bf16 so the norm
                # multiply gets the fast DVE mode
                au = pool.tile([128, 512], BF16, tag="au", bufs=4)
                nc.scalar.copy(au[:], attn_ps[:])
                rec = pool.tile([1, 512], BF16, tag="rec", bufs=4)
                with nc.allow_low_precision(
                        reason="bf16 softmax scale, 2e-2 budget"):
                    nc.vector.reciprocal(rec[:], sums_ps[:])
                recb = pool.tile([128, 512], BF16, tag="recb", bufs=4)
                nc.gpsimd.partition_broadcast(recb[:], rec[:])
                nc.vector.tensor_tensor(attnT[:, h, qsl], au[:], recb[:], mult)

            def outproj_blocks(jc):
                # generator of out-proj (oc, tt) blocks for t-tiles
                # 4jc..4jc+3; drained one block per chain step of the NEXT
                # group so PE never idles while exp paces the chains.
                # PSUM->SBUF copies run on POOL (ACT would head-of-line
                # block the next group's exps)
                tg = slice(jc * 512, (jc + 1) * 512)
                for oc in range(OCH):
                    osl = slice(oc * 512, (oc + 1) * 512)
                    wo = pool.tile([128, ICH, 512], F16, tag="wo", bufs=3)
                    nc.sync.dma_start(wo[:], woutT_d[:, osl].rearrange(
                        "(i p) o -> p i o", p=128))
                    osb = pool.tile([128, 4, 512], F16, tag="osb", bufs=2)
                    for tt in range(4):
                        t = 4 * jc + tt
                        out_ps = psum.tile([128, 512], F32, tag="bank", bufs=4)
                        for i in range(ICH):
                            nc.tensor.matmul(out_ps[:],
                                             attnT[:, i, t * 128:(t + 1) * 128],
                                             wo[:, i, :], start=(i == 0),
                                             stop=(i == ICH - 1))
                        # PSUM->SBUF evac: GPSIMD cannot read PSUM, so
                        # alternate ACT/DVE to halve per-engine load
                        if (oc + tt) % 2 == 0:
                            nc.scalar.copy(osb[:, tt, :], out_ps[:])
                        else:
                            nc.vector.tensor_copy(osb[:, tt, :], out_ps[:])
                        yield
                    nc.sync.dma_start(
                        outp_d[tg, osl].rearrange("(a p) o -> p a o", p=128),
                        osb[:])

            # ---- QKV sweeps, then chain groups with the previous group's
            # out-proj blocks interleaved one-per-step into the PE stream ----
            load_tables()
            for tcx in range(TCH):
                qkv_sweep(tcx)
            op_gen = None
            for jc in range(TCH):
                # spread the previous group's 48 out-proj blocks evenly
                # across this group's chain steps
                steps = HPC * ((4 * jc + 4) // 2 + 1)
                rate = (4 * OCH) / steps if op_gen is not None else 0.0
                due = 0.0
                for h in range(HPC):
                    for _ in attn_chain(h, jc):
                        due += rate
                        while due >= 1.0:
                            next(op_gen, None)
                            due -= 1.0
                if op_gen is not None:
                    for _ in op_gen:
                        pass
                op_gen = outproj_blocks(jc)
            for _ in op_gen:
                pass

    nc.compile()
    return nc


def kernel(hidden_states, position_ids, Wqkv, Wout):
    global _compiled
    hidden_states = np.asarray(hidden_states, dtype=np.float32)
    position_ids = np.asarray(position_ids).astype(np.int64)
    Wqkv = np.asarray(Wqkv, dtype=np.float32)
    Wout = np.asarray(Wout, dtype=np.float32)

    if _compiled is None:
        _compiled = _build()
    nc = _compiled

    # host prep: rope tables (from actual position_ids), masks, shards
    scale = HD ** -0.5
    half = HD // 2
    inv_freq = 1.0 / (THETA ** (np.arange(half, dtype=np.float64) / half))
    freqs = position_ids.astype(np.float64)[None, :] * inv_freq[:, None]  # [64, T]
    cos = np.cos(freqs)
    sin = np.sin(freqs)
    cosf = np.concatenate([cos, cos], 0)
    sinf = np.concatenate([-sin, sin], 0)
    cosq = (cosf * scale).astype(np.float16)
    sinq = (sinf * scale).astype(np.float16)
    cosk = cosf.astype(np.float16)
    sink = sinf.astype(np.float16)

    p = np.arange(128)[:, None]
    f = np.arange(128)[None, :]
    masks = (f >= p).astype(np.float16)

    hidT = np.ascontiguousarray(hidden_states.T).astype(np.float16)

    q_size = N_HEADS * HD
    in_maps = []
    for c in range(N_CORES):
        qrows = Wqkv[c * HPC * HD:(c + 1) * HPC * HD]
        krows = Wqkv[q_size + c * HD:q_size + (c + 1) * HD]
        vrows = Wqkv[q_size + N_KV * HD + c * HD:q_size + N_KV * HD + (c + 1) * HD]
        wqkvT = np.ascontiguousarray(
            np.concatenate([qrows, krows, vrows], 0).T).astype(np.float16)
        woutT = np.ascontiguousarray(
            Wout[:, c * HPC * HD:(c + 1) * HPC * HD].T).astype(np.float16)
        in_maps.append({
            "hidT": hidT, "wqkvT": wqkvT, "woutT": woutT,
            "cosq": cosq, "sinq": sinq, "cosk": cosk, "sink": sink,
            "maskm": masks,
        })

    trace = os.environ.get("DBRX_TRACE", "0") == "1"
    res = run_bass_kernel_spmd(nc, in_maps, core_ids=list(range(N_CORES)),
                               trace=trace)
    kernel.last_result = res

    out = res.results[0]["outp"].astype(np.float32)
    for c in range(1, N_CORES):
        out += res.results[c]["outp"].astype(np.float32)
    return out


# revision 30
# speedup vs baseline: 1.0911x; 1.0062x over previous
r"""DbrxAttention on 8 TRN2 NeuronCores, tensor-parallel across heads.

Per-core shard (core c of 8): 6 query heads (q heads 6c..6c+5), kv head c
(replicated per its 6-head query group), plus the matching 768 input
columns of the out-projection. Each core computes a partial out-proj
(row-parallel Wout); the partials are summed on the host (the all-reduce
of the TP pattern).

Layouts (per core, all device tensors):
  hidT   [6144, 2048] fp16  hidden^T       (d on partitions)
  wqkvT  [6144, 1024] fp16  [q0..q5 | k | v] columns of Wqkv^T shard
  woutT  [768,  6144] fp16  Wout[:, shard]^T
  cos/sin tables [128, 2048] fp16, neox rope with sign-folded sin and the
  1/sqrt(128) score scale folded into the q tables.
  masks  [4, 128, 512] fp16  multiplicative causal masks for the four
         diagonal-straddle patterns of (128-wide kt tile, 512-wide qt chunk)

Pipeline: QKV GEMM (fp16, PSUM fp32) -> clip -> rope (DVE + partition-shift
DMA) into fp16 tiles -> scores^T = k^T.T @ q^T per (head, qt-chunk, kt-tile)
block (fp16 MM, software-pipelined 2 deep) -> exp on ACT into fp32r probs ->
causal mask multiply on diagonal blocks -> softmax row sums accumulated on
DVE (two legs) + one partition-reduce matmul per chain -> attn^T via
v-matmul (fp32r) -> normalization (reciprocal + partition broadcast) ->
fp16 attnT -> out-proj (fp16) interleaved per 4-t-tile group right after
its chains (fills PE while exp paces the chains) -> partial [2048, 6144]
fp16 out, summed across the 8 cores on the host.
"""

import os

import numpy as np

import concourse.mybir as mybir
import concourse.tile as tile
from concourse import bacc
from concourse.bass_utils import run_bass_kernel_spmd

F32R = mybir.dt.float32r
F32 = mybir.dt.float32
F16 = mybir.dt.float16
BF16 = mybir.dt.bfloat16

T = 2048
D = 6144
N_HEADS = 48
N_KV = 8
HD = 128
CLIP = 8.0
THETA = 500000.0
N_CORES = 8
HPC = N_HEADS // N_CORES      # q heads per core = 6
QKJ = HPC + 1                 # q+k j-tiles per core = 7
DCH = D // 128                # 48 contraction chunks
DG = DCH // 4                 # 12 batched (4-chunk) DMA groups
TCH = T // 512                # 4 t-chunks
TTILES = T // 128             # 16 t-tiles
OCH = D // 512                # 12 out-proj column chunks
ICH = HPC                     # 6 out-proj contraction chunks (768/128)

_compiled = None


def _build():
    nc = bacc.Bacc("TRN2", target_bir_lowering=False, debug=False,
                   num_devices=N_CORES)

    hidT_d = nc.dram_tensor("hidT", [D, T], F16, kind="ExternalInput").ap()
    wqkvT_d = nc.dram_tensor("wqkvT", [D, 1024], F16, kind="ExternalInput").ap()
    woutT_d = nc.dram_tensor("woutT", [HPC * HD, D], F16, kind="ExternalInput").ap()
    cosq_d = nc.dram_tensor("cosq", [HD, T], F16, kind="ExternalInput").ap()
    sinq_d = nc.dram_tensor("sinq", [HD, T], F16, kind="ExternalInput").ap()
    cosk_d = nc.dram_tensor("cosk", [HD, T], F16, kind="ExternalInput").ap()
    sink_d = nc.dram_tensor("sink", [HD, T], F16, kind="ExternalInput").ap()
    mask_d = nc.dram_tensor("maskm", [HD, 128], F16, kind="ExternalInput").ap()
    outp_d = nc.dram_tensor("outp", [T, D], F16, kind="ExternalOutput").ap()

    mn, mx = mybir.AluOpType.min, mybir.AluOpType.max
    mult, add = mybir.AluOpType.mult, mybir.AluOpType.add
    EXP = mybir.ActivationFunctionType.Exp

    with tile.TileContext(nc) as tc:
        with (
            tc.tile_pool(name="sb", bufs=1) as pool,
            tc.tile_pool(name="ps", bufs=1, space="PSUM") as psum,
        ):
            # persistent tensors
            qkT = pool.tile([128, QKJ, T], F16)       # roped q (scaled) + k
            v_sb = pool.tile([128, TTILES, HD], BF16)  # clipped v, [t%128, t//128, hd]
            attnT = pool.tile([128, HPC, T], F16)      # normalized attn^T
            cosq = pool.tile([HD, T], F16)
            sinq = pool.tile([HD, T], F16)
            cosk = pool.tile([HD, T], F16)
            sink = pool.tile([HD, T], F16)
            masks = pool.tile([HD, 128], F16)
            ones = pool.tile([HD, 1], BF16)

            def load_tables():
                nc.gpsimd.dma_start(cosq[:], cosq_d[:])
                nc.gpsimd.dma_start(sinq[:], sinq_d[:])
                nc.gpsimd.dma_start(cosk[:], cosk_d[:])
                nc.gpsimd.dma_start(sink[:], sink_d[:])
                nc.gpsimd.dma_start(masks[:], mask_d[:])
                nc.vector.memset(ones[:], 1.0)

            def qkv_sweep(tcx):
                tsl = slice(tcx * 512, (tcx + 1) * 512)
                # PSUM: two 2-bank "wide" tiles hold j-pairs (j01, j23);
                # j4..j6 and v use the 1-bank "bank" ring — 8 banks total,
                # same live set as before but shaped so chains can use a
                # 2-bank sc2 (single 1024-wide exp per 2 kt tiles)
                wides = [psum.tile([128, 1024], F32, tag="wide", bufs=2,
                                   name=f"qkw{w}") for w in range(2)]
                qk_ps = [wides[j // 2][:, (j % 2) * 512:(j % 2 + 1) * 512]
                         for j in range(4)]
                qk_ps += [psum.tile([128, 512], F32, tag="bank", bufs=4,
                                    name=f"qk_ps{j}")[:]
                          for j in range(4, QKJ)]
                v_ps = psum.tile([128, 512], F32, tag="bank", bufs=4)
                for g in range(DG):
                    # batched 4-chunk loads: 4x fewer DMAs on the sync queue.
                    # The very first group loads per-chunk so the first
                    # matmul doesn't wait on the whole 1.5 MB batch.
                    g4 = slice(g * 512, (g + 1) * 512)
                    hid4 = pool.tile([128, 4, 512], F16, tag="hid", bufs=3)
                    wq4 = pool.tile([128, 4, 1024], F16, tag="wq", bufs=3)
                    if tcx == 0 and g == 0:
                        for i in range(4):
                            dsl = slice(i * 128, (i + 1) * 128)
                            nc.sync.dma_start(wq4[:, i, :], wqkvT_d[dsl, :])
                            nc.sync.dma_start(hid4[:, i, :], hidT_d[dsl, tsl])
                    else:
                        nc.sync.dma_start(
                            hid4[:], hidT_d[g4, tsl].rearrange(
                                "(a p) t -> p a t", p=128))
                        nc.sync.dma_start(
                            wq4[:], wqkvT_d[g4, :].rearrange(
                                "(a p) w -> p a w", p=128))
                    for i in range(4):
                        d = g * 4 + i
                        hid = hid4[:, i, :]
                        wq = wq4[:, i, :]
                        st, sp = d == 0, d == DCH - 1
                        for j in range(QKJ):
                            nc.tensor.matmul(qk_ps[j],
                                             wq[:, j * 128:(j + 1) * 128],
                                             hid[:], start=st, stop=sp,
                                             skip_group_check=(j < 4))
                        for s in range(4):
                            # packed quarter-bank outputs: start=True zeroes
                            # the whole 2KB zero-region, so only the first
                            # sub-matmul of the bank may set it
                            nc.tensor.matmul(v_ps[:, s * 128:(s + 1) * 128],
                                             hid[:, s * 128:(s + 1) * 128],
                                             wq[:, 896:1024],
                                             start=(st and s == 0),
                                             stop=(sp and s == 3),
                                             skip_group_check=True)
                # evacuate: all clips first (each clip releases a PSUM bank
                # for the interleaved attention/next-sweep matmuls), ropes
                # after (they only read the SBUF raw tiles)
                raws = []
                for w in range(2):
                    raw2 = pool.tile([128, 1024], F32, tag="raw2", bufs=3,
                                     name=f"raw2_{w}")
                    nc.vector.tensor_scalar(raw2[:], wides[w][:], CLIP, -CLIP,
                                            mn, mx)
                    raws += [raw2[:, 0:512], raw2[:, 512:1024]]
                for j in range(4, QKJ):
                    raw = pool.tile([128, 512], F32, tag="raw", bufs=4,
                                    name=f"raw{j}")
                    nc.vector.tensor_scalar(raw[:], qk_ps[j], CLIP, -CLIP, mn, mx)
                    raws.append(raw[:])
                nc.vector.tensor_scalar(
                    v_sb[:, tcx * 4:(tcx + 1) * 4, :],
                    v_ps[:].rearrange("p (a h) -> p a h", a=4),
                    CLIP, -CLIP, mn, mx)
                # ropes run on POOL: keeps them out of the DVE stream so the
                # first chains' masks/accs aren't head-of-line blocked behind
                # the last sweep's rope tail
                for j in [HPC] + list(range(HPC)):
                    raw = raws[j]
                    xr = pool.tile([128, 512], F32, tag="xr", bufs=4)
                    nc.sync.dma_start(xr[0:64, :], raw[64:128, :])
                    nc.sync.dma_start(xr[64:128, :], raw[0:64, :])
                    cosT = cosq if j < HPC else cosk
                    sinT = sinq if j < HPC else sink
                    dst = qkT[:, j, tsl]
                    nc.gpsimd.tensor_tensor(dst, raw, cosT[:, tsl], mult)
                    nc.gpsimd.tensor_tensor(xr[:], xr[:], sinT[:, tsl], mult)
                    nc.gpsimd.tensor_tensor(dst, dst, xr[:], add)

            def attn_chain(h, jc):
                # generator: yields once per 2-kt block so the driver can
                # interleave ready out-proj matmuls into the in-order PE
                # stream (fills the PE bubble left by the ACT-paced exp).
                # Scores for a kt-pair land in one 2-bank "wide" PSUM tile so
                # a single 1024-wide exp serves both (less ACT overhead).
                # Diagonal-straddle kt tiles (r = kt-4jc >= 0) compute only
                # the causally-needed q-suffix [128r:512] — 15% less
                # score/v PE work; the in-tile triangle is masked by one
                # [128,128] pattern at the suffix head. The unwritten prefix
                # of those PSUM halves holds stale data; exp covers it but
                # nothing downstream reads it.
                qsl = slice(jc * 512, (jc + 1) * 512)
                n_kt = 4 * jc + 4
                n_b = n_kt // 2
                attn_ps = psum.tile([128, 512], F32, tag="bank", bufs=4)
                LEAD = 1
                pbs = {}
                # row sums accumulate on DVE in bf16 (2-byte dtype gets the
                # fast DVE mode); suffix-kt adds land in leg 0 (always fully
                # initialized by kt 0), full-width kts alternate legs
                two_legs = jc >= 1
                accs = [pool.tile([128, 512], BF16, tag=f"acc{i}", bufs=1,
                                  name=f"acc{i}")
                        for i in range(2 if two_legs else 1)]
                accs = accs + accs[:1] if not two_legs else accs
                for bstep in range(n_b + LEAD):
                    if bstep < n_b:
                        b = bstep
                        sc2 = psum.tile([128, 1024], F32, tag="wide", bufs=2)
                        for half in range(2):
                            kt = 2 * b + half
                            r = kt - 4 * jc
                            off = 128 * r if r > 0 else 0
                            nc.tensor.matmul(
                                sc2[:, half * 512 + off:(half + 1) * 512],
                                qkT[:, HPC, kt * 128:(kt + 1) * 128],
                                qkT[:, h, jc * 512 + off:(jc + 1) * 512],
                                start=True, stop=True,
                                skip_group_check=True)
                        pb2 = pool.tile([128, 1024], BF16, tag="pb", bufs=4)
                        nc.scalar.activation(pb2[:], sc2[:], EXP)
                        for half in range(2):
                            kt = 2 * b + half
                            r = kt - 4 * jc
                            if r >= 0:
                                msl = slice(half * 512 + 128 * r,
                                            half * 512 + 128 * r + 128)
                                nc.vector.tensor_tensor(
                                    pb2[:, msl], pb2[:, msl], masks[:], mult)
                        for half in range(2):
                            kt = 2 * b + half
                            r = kt - 4 * jc
                            if r > 0:
                                with nc.allow_low_precision(
                                        reason="bf16 row-sum legs"):
                                    nc.vector.tensor_tensor(
                                        accs[0][:, 128 * r:512],
                                        accs[0][:, 128 * r:512],
                                        pb2[:, half * 512 + 128 * r:
                                            (half + 1) * 512], add)
                            else:
                                leg = accs[kt % 2] if two_legs else accs[0]
                                psl = pb2[:, half * 512:(half + 1) * 512]
                                if kt < 2:
                                    nc.vector.tensor_scalar(
                                        leg[:], psl, 0.0, None, add)
                                else:
                                    with nc.allow_low_precision(
                                            reason="bf16 row-sum legs"):
                                        nc.vector.tensor_tensor(
                                            leg[:], leg[:], psl, add)
                        pbs[b] = pb2
                    if bstep >= LEAD:
                        b = bstep - LEAD
                        pb2 = pbs.pop(b)
                        for half in range(2):
                            kt = 2 * b + half
                            r = kt - 4 * jc
                            off = 128 * r if r > 0 else 0
                            st, sp = kt == 0, kt == n_kt - 1
                            nc.tensor.matmul(
                                attn_ps[:, off:512], v_sb[:, kt, :],
                                pb2[:, half * 512 + off:(half + 1) * 512],
                                start=st, stop=sp, skip_group_check=True)
                    yield
                if two_legs:
                    with nc.allow_low_precision(
                            reason="bf16 row-sum combine, 2e-2 budget"):
                        nc.vector.tensor_tensor(accs[0][:], accs[0][:],
                                                accs[1][:], add)
                sums_ps = psum.tile([1, 512], F32, tag="bank", bufs=4)
                nc.tensor.matmul(sums_ps[:], ones[:, 0:1], accs[0][:],
                                 start=True, stop=True)
                # release the bank fast (ACT copy + DVE recip), then
                # normalize off the critical path; all-bf16 so the norm
                # multiply gets the fast DVE mode
                au = pool.tile([128, 512], BF16, tag="au", bufs=4)
                nc.scalar.copy(au[:], attn_ps[:])
                rec = pool.tile([1, 512], BF16, tag="rec", bufs=4)
                with nc.allow_low_precision(
                        reason="bf16 softmax scale, 2e-2 budget"):
                    nc.vector.reciprocal(rec[:], sums_ps[:])
                recb = pool.tile([128, 512], BF16, tag="recb", bufs=4)
                nc.gpsimd.partition_broadcast(recb[:], rec[:])
                nc.vector.tensor_tensor(attnT[:, h, qsl], au[:], recb[:], mult)

            def outproj_blocks(jc):
                # generator of out-proj (oc, tt) blocks for t-tiles
                # 4jc..4jc+3; drained one block per chain step of the NEXT
                # group so PE never idles while exp paces the chains.
                # PSUM->SBUF copies run on POOL (ACT would head-of-line
                # block the next group's exps)
                tg = slice(jc * 512, (jc + 1) * 512)
                for oc in range(OCH):
                    osl = slice(oc * 512, (oc + 1) * 512)
                    wo = pool.tile([128, ICH, 512], F16, tag="wo", bufs=3)
                    nc.sync.dma_start(wo[:], woutT_d[:, osl].rearrange(
                        "(i p) o -> p i o", p=128))
                    osb = pool.tile([128, 4, 512], F16, tag="osb", bufs=2)
                    for tt in range(4):
                        t = 4 * jc + tt
                        out_ps = psum.tile([128, 512], F32, tag="bank", bufs=4)
                        for i in range(ICH):
                            nc.tensor.matmul(out_ps[:],
                                             attnT[:, i, t * 128:(t + 1) * 128],
                                             wo[:, i, :], start=(i == 0),
                                             stop=(i == ICH - 1))
                        # PSUM->SBUF evac: GPSIMD cannot read PSUM, so
                        # alternate ACT/DVE to halve per-engine load
                        if (oc + tt) % 2 == 0:
                            nc.scalar.copy(osb[:, tt, :], out_ps[:])
                        else:
                            nc.vector.tensor_copy(osb[:, tt, :], out_ps[:])
                        if tt % 2 == 1:
                            th = slice(jc * 512 + (tt - 1) * 128,
                                       jc * 512 + (tt + 1) * 128)
                            nc.sync.dma_start(
                                outp_d[th, osl].rearrange(
                                    "(a p) o -> p a o", p=128),
                                osb[:, tt - 1:tt + 1, :])
                        yield

            # ---- QKV sweeps, then chain groups with the previous group's
            # out-proj blocks interleaved one-per-step into the PE stream ----
            load_tables()
            for tcx in range(TCH):
                qkv_sweep(tcx)
            op_gen = None
            for jc in range(TCH):
                # spread the previous group's 48 out-proj blocks evenly
                # across this group's chain steps
                steps = HPC * ((4 * jc + 4) // 2 + 1)
                rate = (4 * OCH) / steps if op_gen is not None else 0.0
                due = 0.0
                for h in range(HPC):
                    for _ in attn_chain(h, jc):
                        due += rate
                        while due >= 1.0:
                            next(op_gen, None)
                            due -= 1.0
                if op_gen is not None:
                    for _ in op_gen:
                        pass
                op_gen = outproj_blocks(jc)
            for _ in op_gen:
                pass

    nc.compile()
    return nc


def kernel(hidden_states, position_ids, Wqkv, Wout):
    global _compiled
    hidden_states = np.asarray(hidden_states, dtype=np.float32)
    position_ids = np.asarray(position_ids).astype(np.int64)
    Wqkv = np.asarray(Wqkv, dtype=np.float32)
    Wout = np.asarray(Wout, dtype=np.float32)

    if _compiled is None:
        _compiled = _build()
    nc = _compiled

    # host prep: rope tables (from actual position_ids), masks, shards
    scale = HD ** -0.5
    half = HD // 2
    inv_freq = 1.0 / (THETA ** (np.arange(half, dtype=np.float64) / half))
    freqs = position_ids.astype(np.float64)[None, :] * inv_freq[:, None]  # [64, T]
    cos = np.cos(freqs)
    sin = np.sin(freqs)
    cosf = np.concatenate([cos, cos], 0)
    sinf = np.concatenate([-sin, sin], 0)
    cosq = (cosf * scale).astype(np.float16)
    sinq = (sinf * scale).astype(np.float16)
    cosk = cosf.astype(np.float16)
    sink = sinf.astype(np.float16)

    p = np.arange(128)[:, None]
    f = np.arange(128)[None, :]
    masks = (f >= p).astype(np.float16)

    hidT = np.ascontiguousarray(hidden_states.T).astype(np.float16)

    q_size = N_HEADS * HD
    in_maps = []
    for c in range(N_CORES):
        qrows = Wqkv[c * HPC * HD:(c + 1) * HPC * HD]
        krows = Wqkv[q_size + c * HD:q_size + (c + 1) * HD]
        vrows = Wqkv[q_size + N_KV * HD + c * HD:q_size + N_KV * HD + (c + 1) * HD]
        wqkvT = np.ascontiguousarray(
            np.concatenate([qrows, krows, vrows], 0).T).astype(np.float16)
        woutT = np.ascontiguousarray(
            Wout[:, c * HPC * HD:(c + 1) * HPC * HD].T).astype(np.float16)
        in_maps.append({
            "hidT": hidT, "wqkvT": wqkvT, "woutT": woutT,
            "cosq": cosq, "sinq": sinq, "cosk": cosk, "sink": sink,
            "maskm": masks,
        })

    trace = os.environ.get("DBRX_TRACE", "0") == "1"
    res = run_bass_kernel_spmd(nc, in_maps, core_ids=list(range(N_CORES)),
                               trace=trace)
    kernel.last_result = res

    out = res.results[0]["outp"].astype(np.float32)
    for c in range(1, N_CORES):
        out += res.results[c]["outp"].astype(np.float32)
    return out


# revision 31
# speedup vs baseline: 1.1270x; 1.0329x over previous
r"""DbrxAttention on 8 TRN2 NeuronCores, tensor-parallel across heads.

Per-core shard (core c of 8): 6 query heads (q heads 6c..6c+5), kv head c
(replicated per its 6-head query group), plus the matching 768 input
columns of the out-projection. Each core computes a partial out-proj
(row-parallel Wout); the partials are summed on the host (the all-reduce
of the TP pattern).

Layouts (per core, all device tensors):
  hidT   [6144, 2048] fp16  hidden^T       (d on partitions)
  wqkvT  [6144, 1024] fp16  [q0..q5 | k | v] columns of Wqkv^T shard
  woutT  [768,  6144] fp16  Wout[:, shard]^T
  cos/sin tables [128, 2048] fp16, neox rope with sign-folded sin and the
  1/sqrt(128) score scale folded into the q tables.
  masks  [4, 128, 512] fp16  multiplicative causal masks for the four
         diagonal-straddle patterns of (128-wide kt tile, 512-wide qt chunk)

Pipeline: QKV GEMM (fp16, PSUM fp32) -> clip -> rope (DVE + partition-shift
DMA) into fp16 tiles -> scores^T = k^T.T @ q^T per (head, qt-chunk, kt-tile)
block (fp16 MM, software-pipelined 2 deep) -> exp on ACT into fp32r probs ->
causal mask multiply on diagonal blocks -> softmax row sums accumulated on
DVE (two legs) + one partition-reduce matmul per chain -> attn^T via
v-matmul (fp32r) -> normalization (reciprocal + partition broadcast) ->
fp16 attnT -> out-proj (fp16) interleaved per 4-t-tile group right after
its chains (fills PE while exp paces the chains) -> partial [2048, 6144]
fp16 out, summed across the 8 cores on the host.
"""

import os

import numpy as np

import concourse.mybir as mybir
import concourse.tile as tile
from concourse import bacc
from concourse.bass_utils import run_bass_kernel_spmd

F32R = mybir.dt.float32r
F32 = mybir.dt.float32
F16 = mybir.dt.float16
BF16 = mybir.dt.bfloat16

T = 2048
D = 6144
N_HEADS = 48
N_KV = 8
HD = 128
CLIP = 8.0
THETA = 500000.0
N_CORES = 8
HPC = N_HEADS // N_CORES      # q heads per core = 6
QKJ = HPC + 1                 # q+k j-tiles per core = 7
DCH = D // 128                # 48 contraction chunks
DG = DCH // 4                 # 12 batched (4-chunk) DMA groups
TCH = T // 512                # 4 t-chunks
TTILES = T // 128             # 16 t-tiles
OCH = D // 512                # 12 out-proj column chunks
ICH = HPC                     # 6 out-proj contraction chunks (768/128)

_compiled = None


def _build():
    nc = bacc.Bacc("TRN2", target_bir_lowering=False, debug=False,
                   num_devices=N_CORES)

    hidT_d = nc.dram_tensor("hidT", [D, T], F16, kind="ExternalInput").ap()
    wqkvT_d = nc.dram_tensor("wqkvT", [D, 1024], F16, kind="ExternalInput").ap()
    woutT_d = nc.dram_tensor("woutT", [HPC * HD, D], F16, kind="ExternalInput").ap()
    cosq_d = nc.dram_tensor("cosq", [HD, T], F16, kind="ExternalInput").ap()
    sinq_d = nc.dram_tensor("sinq", [HD, T], F16, kind="ExternalInput").ap()
    cosk_d = nc.dram_tensor("cosk", [HD, T], F16, kind="ExternalInput").ap()
    sink_d = nc.dram_tensor("sink", [HD, T], F16, kind="ExternalInput").ap()
    mask_d = nc.dram_tensor("maskm", [HD, 128], F16, kind="ExternalInput").ap()
    outp_d = nc.dram_tensor("outp", [T, D], F16, kind="ExternalOutput").ap()

    mn, mx = mybir.AluOpType.min, mybir.AluOpType.max
    mult, add = mybir.AluOpType.mult, mybir.AluOpType.add
    EXP = mybir.ActivationFunctionType.Exp

    with tile.TileContext(nc) as tc:
        with (
            tc.tile_pool(name="sb", bufs=1) as pool,
            tc.tile_pool(name="ps", bufs=1, space="PSUM") as psum,
        ):
            # persistent tensors
            qkT = pool.tile([128, QKJ, T], F16)       # roped q (scaled) + k
            v_sb = pool.tile([128, TTILES, HD], BF16)  # clipped v, [t%128, t//128, hd]
            attnT = pool.tile([128, HPC, T], F16)      # normalized attn^T
            cosq = pool.tile([HD, T], F16)
            sinq = pool.tile([HD, T], F16)
            cosk = pool.tile([HD, T], F16)
            sink = pool.tile([HD, T], F16)
            masks = pool.tile([HD, 128], F16)
            ones = pool.tile([HD, 1], BF16)
            # persistent hid slab: per-sweep writes overwrite slices, so the
            # WAR against the previous sweep's readers is tracked per-slice
            # (a per-sweep pool.tile would bump the whole-tile version and
            # serialize the refill behind all of pass B)
            hslab = pool.tile([128, DCH, 512], F16)

            def load_tables():
                nc.gpsimd.dma_start(cosq[:], cosq_d[:])
                nc.gpsimd.dma_start(sinq[:], sinq_d[:])
                nc.gpsimd.dma_start(cosk[:], cosk_d[:])
                nc.gpsimd.dma_start(sink[:], sink_d[:])
                nc.gpsimd.dma_start(masks[:], mask_d[:])
                nc.vector.memset(ones[:], 1.0)

            def qkv_sweep(tcx, interleave=None):
                # Pass A: q heads j0..j5 accumulate in three 2-bank "wide"
                # PSUM tiles (2 from the sc2 ring + the attnw tile) over all
                # 48 d-chunks; hid lands in a resident SBUF slab. Pass B:
                # k (j6) and v accumulate in the 2-bank ring re-reading the
                # slab (no second hid DMA). During pass B the wides are free
                # again, so the previous chunk's attention chains interleave
                # into the PE stream here (hiding the chain latency that
                # otherwise pays off only after the last sweep).
                tsl = slice(tcx * 512, (tcx + 1) * 512)
                widesA = [psum.tile([128, 1024], F32, tag="wide", bufs=2,
                                    name=f"qkw{w}") for w in range(2)]
                widesA.append(psum.tile([128, 1024], F32, tag="attnw", bufs=1,
                                        name="qkw2"))
                qk_ps = [widesA[j // 2][:, (j % 2) * 512:(j % 2 + 1) * 512]
                         for j in range(6)]
                for g in range(DG):
                    g4 = slice(g * 512, (g + 1) * 512)
                    wqa = pool.tile([128, 4, 768], F16, tag="wqa", bufs=2)
                    if tcx == 0 and g == 0:
                        for i in range(4):
                            dsl = slice(i * 128, (i + 1) * 128)
                            nc.sync.dma_start(wqa[:, i, :],
                                              wqkvT_d[dsl, 0:768])
                            nc.scalar.dma_start(hslab[:, i, :],
                                                hidT_d[dsl, tsl])
                    else:
                        nc.scalar.dma_start(
                            hslab[:, g * 4:(g + 1) * 4, :],
                            hidT_d[g4, tsl].rearrange("(a p) t -> p a t",
                                                      p=128))
                        nc.sync.dma_start(
                            wqa[:], wqkvT_d[g4, 0:768].rearrange(
                                "(a p) w -> p a w", p=128))
                    for i in range(4):
                        d = g * 4 + i
                        st, sp = d == 0, d == DCH - 1
                        for j in range(6):
                            nc.tensor.matmul(qk_ps[j],
                                             wqa[:, i, j * 128:(j + 1) * 128],
                                             hslab[:, d, :], start=st,
                                             stop=sp, skip_group_check=True)
                # evac A: clips first (release the wides for the interleaved
                # chains), then ropes for q0..q5 on POOL
                rawsq = []
                for w in range(3):
                    raw2 = pool.tile([128, 1024], F32, tag="raw2", bufs=3,
                                     name=f"raw2_{w}")
                    nc.vector.tensor_scalar(raw2[:], widesA[w][:], CLIP, -CLIP,
                                            mn, mx)
                    rawsq += [raw2[:, 0:512], raw2[:, 512:1024]]

                def rope(j, raw):
                    xr = pool.tile([128, 512], F32, tag="xr", bufs=4)
                    nc.sync.dma_start(xr[0:64, :], raw[64:128, :])
                    nc.sync.dma_start(xr[64:128, :], raw[0:64, :])
                    cosT = cosq if j < HPC else cosk
                    sinT = sinq if j < HPC else sink
                    dst = qkT[:, j, tsl]
                    nc.gpsimd.tensor_tensor(dst, raw, cosT[:, tsl], mult)
                    nc.gpsimd.tensor_tensor(xr[:], xr[:], sinT[:, tsl], mult)
                    nc.gpsimd.tensor_tensor(dst, dst, xr[:], add)

                for j in range(6):
                    rope(j, rawsq[j])
                # pass B: k and v from the slab; previous chunk's chains
                # interleave here
                k_ps = psum.tile([128, 512], F32, tag="bank", bufs=2)
                v_ps = psum.tile([128, 512], F32, tag="bank", bufs=2)
                due = 0.0
                n_y = HPC * ((4 * (tcx - 1) + 4) // 2 + 1) if tcx >= 1 else 0
                rate = n_y / DCH if interleave is not None else 0.0
                for g in range(DG):
                    g4 = slice(g * 512, (g + 1) * 512)
                    wqb = pool.tile([128, 4, 256], F16, tag="wqb", bufs=2)
                    nc.sync.dma_start(
                        wqb[:], wqkvT_d[g4, 768:1024].rearrange(
                            "(a p) w -> p a w", p=128))
                    for i in range(4):
                        d = g * 4 + i
                        st, sp = d == 0, d == DCH - 1
                        nc.tensor.matmul(k_ps[:], wqb[:, i, 0:128],
                                         hslab[:, d, :], start=st, stop=sp)
                        for s in range(4):
                            # packed quarter-bank outputs: start=True zeroes
                            # the whole 2KB zero-region, so only the first
                            # sub-matmul of the bank may set it
                            nc.tensor.matmul(v_ps[:, s * 128:(s + 1) * 128],
                                             hslab[:, d, s * 128:(s + 1) * 128],
                                             wqb[:, i, 128:256],
                                             start=(st and s == 0),
                                             stop=(sp and s == 3),
                                             skip_group_check=True)
                        due += rate
                        while due >= 1.0:
                            next(interleave, None)
                            due -= 1.0
                if interleave is not None:
                    for _ in interleave:
                        pass
                # evac B
                rawk = pool.tile([128, 512], F32, tag="raw", bufs=2)
                nc.vector.tensor_scalar(rawk[:], k_ps[:], CLIP, -CLIP, mn, mx)
                nc.vector.tensor_scalar(
                    v_sb[:, tcx * 4:(tcx + 1) * 4, :],
                    v_ps[:].rearrange("p (a h) -> p a h", a=4),
                    CLIP, -CLIP, mn, mx)
                rope(HPC, rawk[:])

            def attn_chain(h, jc):
                # generator: yields once per 2-kt block so the driver can
                # interleave ready out-proj matmuls into the in-order PE
                # stream (fills the PE bubble left by the ACT-paced exp).
                # Scores for a kt-pair land in one 2-bank "wide" PSUM tile so
                # a single 1024-wide exp serves both (less ACT overhead).
                # Diagonal-straddle kt tiles (r = kt-4jc >= 0) compute only
                # the causally-needed q-suffix [128r:512] — 15% less
                # score/v PE work; the in-tile triangle is masked by one
                # [128,128] pattern at the suffix head. The unwritten prefix
                # of those PSUM halves holds stale data; exp covers it but
                # nothing downstream reads it.
                qsl = slice(jc * 512, (jc + 1) * 512)
                n_kt = 4 * jc + 4
                n_b = n_kt // 2
                attnw = psum.tile([128, 1024], F32, tag="attnw", bufs=1)
                attn_ps = attnw[:, 0:512]
                LEAD = 1
                pbs = {}
                # row sums accumulate on DVE in bf16 (2-byte dtype gets the
                # fast DVE mode); suffix-kt adds land in leg 0 (always fully
                # initialized by kt 0), full-width kts alternate legs
                two_legs = jc >= 1
                accs = [pool.tile([128, 512], BF16, tag=f"acc{i}", bufs=1,
                                  name=f"acc{i}")
                        for i in range(2 if two_legs else 1)]
                accs = accs + accs[:1] if not two_legs else accs
                for bstep in range(n_b + LEAD):
                    if bstep < n_b:
                        b = bstep
                        sc2 = psum.tile([128, 1024], F32, tag="wide", bufs=2)
                        for half in range(2):
                            kt = 2 * b + half
                            r = kt - 4 * jc
                            off = 128 * r if r > 0 else 0
                            nc.tensor.matmul(
                                sc2[:, half * 512 + off:(half + 1) * 512],
                                qkT[:, HPC, kt * 128:(kt + 1) * 128],
                                qkT[:, h, jc * 512 + off:(jc + 1) * 512],
                                start=True, stop=True,
                                skip_group_check=True)
                        pb2 = pool.tile([128, 1024], BF16, tag="pb", bufs=4)
                        nc.scalar.activation(pb2[:], sc2[:], EXP)
                        for half in range(2):
                            kt = 2 * b + half
                            r = kt - 4 * jc
                            if r >= 0:
                                msl = slice(half * 512 + 128 * r,
                                            half * 512 + 128 * r + 128)
                                nc.vector.tensor_tensor(
                                    pb2[:, msl], pb2[:, msl], masks[:], mult)
                        for half in range(2):
                            kt = 2 * b + half
                            r = kt - 4 * jc
                            if r > 0:
                                with nc.allow_low_precision(
                                        reason="bf16 row-sum legs"):
                                    nc.vector.tensor_tensor(
                                        accs[0][:, 128 * r:512],
                                        accs[0][:, 128 * r:512],
                                        pb2[:, half * 512 + 128 * r:
                                            (half + 1) * 512], add)
                            else:
                                leg = accs[kt % 2] if two_legs else accs[0]
                                psl = pb2[:, half * 512:(half + 1) * 512]
                                if kt < 2:
                                    nc.vector.tensor_scalar(
                                        leg[:], psl, 0.0, None, add)
                                else:
                                    with nc.allow_low_precision(
                                            reason="bf16 row-sum legs"):
                                        nc.vector.tensor_tensor(
                                            leg[:], leg[:], psl, add)
                        pbs[b] = pb2
                    if bstep >= LEAD:
                        b = bstep - LEAD
                        pb2 = pbs.pop(b)
                        for half in range(2):
                            kt = 2 * b + half
                            r = kt - 4 * jc
                            off = 128 * r if r > 0 else 0
                            st, sp = kt == 0, kt == n_kt - 1
                            nc.tensor.matmul(
                                attnw[:, off:512], v_sb[:, kt, :],
                                pb2[:, half * 512 + off:(half + 1) * 512],
                                start=st, stop=sp, skip_group_check=True)
                    yield
                if two_legs:
                    with nc.allow_low_precision(
                            reason="bf16 row-sum combine, 2e-2 budget"):
                        nc.vector.tensor_tensor(accs[0][:], accs[0][:],
                                                accs[1][:], add)
                sums_ps = attnw[0:1, 512:1024]
                nc.tensor.matmul(sums_ps, ones[:, 0:1], accs[0][:],
                                 start=True, stop=True)
                # release the bank fast (ACT copy + DVE recip), then
                # normalize off the critical path; all-bf16 so the norm
                # multiply gets the fast DVE mode
                au = pool.tile([128, 512], BF16, tag="au", bufs=4)
                nc.scalar.copy(au[:], attnw[:, 0:512])
                rec = pool.tile([1, 512], BF16, tag="rec", bufs=4)
                with nc.allow_low_precision(
                        reason="bf16 softmax scale, 2e-2 budget"):
                    nc.vector.reciprocal(rec[:], sums_ps)
                recb = pool.tile([128, 512], BF16, tag="recb", bufs=4)
                nc.gpsimd.partition_broadcast(recb[:], rec[:])
                nc.vector.tensor_tensor(attnT[:, h, qsl], au[:], recb[:], mult)

            def outproj_blocks(jc):
                # generator of out-proj (oc, tt) blocks for t-tiles
                # 4jc..4jc+3; drained one block per chain step of the NEXT
                # group so PE never idles while exp paces the chains.
                # PSUM->SBUF copies run on POOL (ACT would head-of-line
                # block the next group's exps)
                tg = slice(jc * 512, (jc + 1) * 512)
                for oc in range(OCH):
                    osl = slice(oc * 512, (oc + 1) * 512)
                    wo = pool.tile([128, ICH, 512], F16, tag="wo", bufs=3)
                    nc.sync.dma_start(wo[:], woutT_d[:, osl].rearrange(
                        "(i p) o -> p i o", p=128))
                    osb = pool.tile([128, 4, 512], F16, tag="osb", bufs=2)
                    for tt in range(4):
                        t = 4 * jc + tt
                        out_ps = psum.tile([128, 512], F32, tag="bank", bufs=2)
                        for i in range(ICH):
                            nc.tensor.matmul(out_ps[:],
                                             attnT[:, i, t * 128:(t + 1) * 128],
                                             wo[:, i, :], start=(i == 0),
                                             stop=(i == ICH - 1))
                        # PSUM->SBUF evac: GPSIMD cannot read PSUM, so
                        # alternate ACT/DVE to halve per-engine load
                        if (oc + tt) % 2 == 0:
                            nc.scalar.copy(osb[:, tt, :], out_ps[:])
                        else:
                            nc.vector.tensor_copy(osb[:, tt, :], out_ps[:])
                        if tt % 2 == 1:
                            th = slice(jc * 512 + (tt - 1) * 128,
                                       jc * 512 + (tt + 1) * 128)
                            nc.sync.dma_start(
                                outp_d[th, osl].rearrange(
                                    "(a p) o -> p a o", p=128),
                                osb[:, tt - 1:tt + 1, :])
                        yield

            # ---- Sweeps with the previous chunk's chains interleaved
            # into pass B; post-QKV: last chunk's chains with all out-proj
            # groups interleaved into the PE stream ----
            def chain_group(jc):
                for h in range(HPC):
                    for _ in attn_chain(h, jc):
                        yield

            load_tables()
            qkv_sweep(0)
            for tcx in range(1, TCH):
                qkv_sweep(tcx, interleave=chain_group(tcx - 1))
            op_gens = [outproj_blocks(jc) for jc in range(TCH - 1)]

            def ops_ready():
                for gch in op_gens:
                    for _ in gch:
                        yield

            ops = ops_ready()
            due = 0.0
            rate = (3 * 4 * OCH) / (HPC * ((4 * 3 + 4) // 2 + 1))
            for h in range(HPC):
                for _ in attn_chain(h, TCH - 1):
                    due += rate
                    while due >= 1.0:
                        next(ops, None)
                        due -= 1.0
            for _ in ops:
                pass
            for _ in outproj_blocks(TCH - 1):
                pass

    nc.compile()
    return nc


def kernel(hidden_states, position_ids, Wqkv, Wout):
    global _compiled
    hidden_states = np.asarray(hidden_states, dtype=np.float32)
    position_ids = np.asarray(position_ids).astype(np.int64)
    Wqkv = np.asarray(Wqkv, dtype=np.float32)
    Wout = np.asarray(Wout, dtype=np.float32)

    if _compiled is None:
        _compiled = _build()
    nc = _compiled

    # host prep: rope tables (from actual position_ids), masks, shards
    scale = HD ** -0.5
    half = HD // 2
    inv_freq = 1.0 / (THETA ** (np.arange(half, dtype=np.float64) / half))
    freqs = position_ids.astype(np.float64)[None, :] * inv_freq[:, None]  # [64, T]
    cos = np.cos(freqs)
    sin = np.sin(freqs)
    cosf = np.concatenate([cos, cos], 0)
    sinf = np.concatenate([-sin, sin], 0)
    cosq = (cosf * scale).astype(np.float16)
    sinq = (sinf * scale).astype(np.float16)
    cosk = cosf.astype(np.float16)
    sink = sinf.astype(np.float16)

    p = np.arange(128)[:, None]
    f = np.arange(128)[None, :]
    masks = (f >= p).astype(np.float16)

    hidT = np.ascontiguousarray(hidden_states.T).astype(np.float16)

    q_size = N_HEADS * HD
    in_maps = []
    for c in range(N_CORES):
        qrows = Wqkv[c * HPC * HD:(c + 1) * HPC * HD]
        krows = Wqkv[q_size + c * HD:q_size + (c + 1) * HD]
        vrows = Wqkv[q_size + N_KV * HD + c * HD:q_size + N_KV * HD + (c + 1) * HD]
        wqkvT = np.ascontiguousarray(
            np.concatenate([qrows, krows, vrows], 0).T).astype(np.float16)
        woutT = np.ascontiguousarray(
            Wout[:, c * HPC * HD:(c + 1) * HPC * HD].T).astype(np.float16)
        in_maps.append({
            "hidT": hidT, "wqkvT": wqkvT, "woutT": woutT,
            "cosq": cosq, "sinq": sinq, "cosk": cosk, "sink": sink,
            "maskm": masks,
        })

    trace = os.environ.get("DBRX_TRACE", "0") == "1"
    res = run_bass_kernel_spmd(nc, in_maps, core_ids=list(range(N_CORES)),
                               trace=trace)
    kernel.last_result = res

    out = res.results[0]["outp"].astype(np.float32)
    for c in range(1, N_CORES):
        out += res.results[c]["outp"].astype(np.float32)
    return out
